# revision 9
# baseline (speedup 1.0000x reference)
"""Class-conditional VQ-VAE forward pass on 8 Trainium2 NeuronCores.

Strategy (data-parallel over batch, per spec sharding_hint):
  - Host shards the batch 8 ways, passes each core x^T (transposed shard) +
    replicated weights + E^T.
  - Device per core:
      * builds the full decoder lookup table X[1024, 2000] = decode(E) once
        (every batch row's decoder output is a function of its code only,
        since z_st == z_q numerically in the forward pass),
      * runs the encoder MLP in fp32 as chained transposed matmuls
        (zero on-chip transposes),
      * VQ argmin via scores s = 2*z.e - |e|^2 (one matmul with an appended
        ones-row; argmax == argmin of distances) + Max8/MaxIndex,
      * x_rec rows come from an indirect-DMA gather of X by code index.
  - Host assembles shards and computes the two scalar losses from
    z_e / code_idx (tiny reductions).
All matmuls are fp32 so code_idx matches the fp32 reference argmin exactly.
"""

import os
import numpy as np

B, IN, H1, H2, LAT, KCODES = 32768, 2000, 512, 256, 32, 1024
NCORES = 8
S = B // NCORES          # 4096 rows per core
BLK = 512                # rows per pipeline block
NBLK = S // BLK          # 8
NRT = S // 128           # 32 row-tiles per core
P = 128
KC1 = (IN + P - 1) // P  # 16 chunks over IN (15*128 + 80)
KC1_LAST = IN - (KC1 - 1) * P  # 80
KC2 = H1 // P            # 4
KC3 = H2 // P            # 2
MC1 = H1 // P            # 4 output chunks of L1
MC2 = H2 // P            # 2 output chunks of L2
NSC = KCODES // 512      # 2 score column chunks
NXC = 4                  # X free-dim chunks of 500
XCW = IN // NXC          # 500

_CACHED = {}


def _build_nc():
    import concourse.tile as tile
    import concourse.mybir as mybir
    from concourse import bacc

    f32 = mybir.dt.float32
    u32 = mybir.dt.uint32

    nc = bacc.Bacc(trn_type="TRN2")

    # ---- DRAM I/O ----
    xT = nc.dram_tensor("xT", [IN, S], f32, kind="ExternalInput")
    We1 = nc.dram_tensor("We1", [IN, H1], f32, kind="ExternalInput")
    be1 = nc.dram_tensor("be1", [H1], f32, kind="ExternalInput")
    We2 = nc.dram_tensor("We2", [H1, H2], f32, kind="ExternalInput")
    be2 = nc.dram_tensor("be2", [H2], f32, kind="ExternalInput")
    We3 = nc.dram_tensor("We3", [H2, LAT], f32, kind="ExternalInput")
    be3 = nc.dram_tensor("be3", [LAT], f32, kind="ExternalInput")
    Wd1 = nc.dram_tensor("Wd1", [LAT, H2], f32, kind="ExternalInput")
    bd1 = nc.dram_tensor("bd1", [H2], f32, kind="ExternalInput")
    Wd2 = nc.dram_tensor("Wd2", [H2, H1], f32, kind="ExternalInput")
    bd2 = nc.dram_tensor("bd2", [H1], f32, kind="ExternalInput")
    Wd3 = nc.dram_tensor("Wd3", [H1, IN], f32, kind="ExternalInput")
    bd3 = nc.dram_tensor("bd3", [IN], f32, kind="ExternalInput")
    ETd = nc.dram_tensor("ET", [LAT, KCODES], f32, kind="ExternalInput")

    x_rec = nc.dram_tensor("x_rec", [S, IN], f32, kind="ExternalOutput")
    code_o = nc.dram_tensor("code", [S], u32, kind="ExternalOutput")
    zeT_o = nc.dram_tensor("zeT", [LAT, S], f32, kind="ExternalOutput")

    Xd = nc.dram_tensor("Xtab", [KCODES, IN], f32, kind="Internal")

    Relu = mybir.ActivationFunctionType.Relu
    Copy = mybir.ActivationFunctionType.Copy
    from concourse.bass import IndirectOffsetOnAxis

    with tile.TileContext(nc) as tc:
        with (
            tc.tile_pool(name="wshare", bufs=1) as pws,   # we1/wd3 shared slot
            tc.tile_pool(name="wsmall", bufs=1) as pw,
            tc.tile_pool(name="dec", bufs=1) as pdec,
            tc.tile_pool(name="xstage", bufs=1) as pxs,
            tc.tile_pool(name="xt", bufs=2) as pxt,
            tc.tile_pool(name="act", bufs=2) as pact,
            tc.tile_pool(name="scr", bufs=2) as psc,
            tc.tile_pool(name="small", bufs=1) as psm,
            tc.tile_pool(name="gat", bufs=2) as pg,
            tc.tile_pool(name="ps_h1", bufs=2, space="PSUM") as ps_h1,
            tc.tile_pool(name="ps_h2", bufs=1, space="PSUM") as ps_h2,
            tc.tile_pool(name="ps_z", bufs=1, space="PSUM") as ps_z,
            tc.tile_pool(name="ps_sc", bufs=2, space="PSUM") as ps_sc,
            tc.tile_pool(name="ps_m", bufs=2, space="PSUM") as ps_m,
        ):
            # ---------------- weight/bias loads (small ones) -------------
            we2_sb = pw.tile([P, KC2, H2], f32, tag="we2")
            nc.sync.dma_start(we2_sb[:], We2[:].rearrange("(c p) m -> p c m", p=P))
            we3_sb = pw.tile([P, KC3, LAT], f32, tag="we3")
            nc.sync.dma_start(we3_sb[:], We3[:].rearrange("(c p) m -> p c m", p=P))
            wd1_sb = pw.tile([LAT, H2], f32, tag="wd1")
            nc.sync.dma_start(wd1_sb[:], Wd1[:])
            wd2_sb = pw.tile([P, KC3, H1], f32, tag="wd2")
            nc.sync.dma_start(wd2_sb[:], Wd2[:].rearrange("(c p) m -> p c m", p=P))

            bias_sb = pw.tile([P, 16], f32, tag="bias")
            nc.sync.dma_start(bias_sb[:, 0:4], be1[:].rearrange("(c p) -> p c", p=P))
            nc.sync.dma_start(bias_sb[:, 4:6], be2[:].rearrange("(c p) -> p c", p=P))
            nc.sync.dma_start(bias_sb[0:LAT, 6:7], be3[:, None])
            nc.sync.dma_start(bias_sb[:, 7:9], bd1[:].rearrange("(c p) -> p c", p=P))
            nc.sync.dma_start(bias_sb[:, 9:13], bd2[:].rearrange("(c p) -> p c", p=P))
            bd3_sb = pw.tile([1, IN], f32, tag="bd3")
            nc.sync.dma_start(bd3_sb[:], bd3[None, :])

            et_sb = pw.tile([LAT, KCODES], f32, tag="et")
            nc.sync.dma_start(et_sb[:], ETd[:])

            # ---------------- VQ score matrix  E2t = [2*E^T ; -|e|^2] ----
            e2t = pw.tile([64, KCODES], f32, tag="e2t")
            nc.vector.tensor_scalar_mul(e2t[0:LAT, :], et_sb[:], 2.0)
            ones32 = pw.tile([LAT, 1], f32, tag="ones")
            nc.gpsimd.memset(ones32[:], 1.0)
            ones_row = pw.tile([1, P], f32, tag="onesr")
            nc.gpsimd.memset(ones_row[:], 1.0)
            # squared codebook: use d1t[0] rows 0:32 as scratch (WAR-safe,
            # D1T is written only after the colsum matmuls below read it)
            d1t = [pdec.tile([P, KCODES], f32, tag=f"d1_{m}", name=f"d1_{m}") for m in range(MC2)]
            sq = d1t[0][0:LAT, :]
            nc.vector.tensor_mul(sq, et_sb[:], et_sb[:])
            for n in range(NSC):
                psn = ps_m.tile([P, 512], f32, tag="psm")
                nc.tensor.matmul(
                    psn[0:1, :], ones32[:], sq[:, n * 512:(n + 1) * 512],
                    start=True, stop=True,
                )
                nc.scalar.activation(
                    e2t[LAT:LAT + 1, n * 512:(n + 1) * 512], psn[0:1, :],
                    Copy, scale=-1.0,
                )

            # ---------------- decoder table X = decode(E) ----------------
            # D1T[h2, code] = relu(Wd1^T @ E^T + bd1)
            for m in range(MC2):
                for n in range(NSC):
                    psn = ps_m.tile([P, 512], f32, tag="psm")
                    nc.tensor.matmul(
                        psn[:], wd1_sb[:, m * P:(m + 1) * P],
                        et_sb[:, n * 512:(n + 1) * 512],
                        start=True, stop=True,
                    )
                    nc.scalar.activation(
                        d1t[m][:, n * 512:(n + 1) * 512], psn[:], Relu,
                        bias=bias_sb[:, 7 + m:8 + m],
                    )
            # D2T[h1, code] = relu(Wd2^T @ D1T + bd2)
            d2t = [pdec.tile([P, KCODES], f32, tag=f"d2_{m}", name=f"d2_{m}") for m in range(MC1)]
            for m in range(MC1):
                for n in range(NSC):
                    psn = ps_m.tile([P, 512], f32, tag="psm")
                    for k in range(KC3):
                        nc.tensor.matmul(
                            psn[:], wd2_sb[:, k, m * P:(m + 1) * P],
                            d1t[k][:, n * 512:(n + 1) * 512],
                            start=(k == 0), stop=(k == KC3 - 1),
                        )
                    nc.scalar.activation(
                        d2t[m][:, n * 512:(n + 1) * 512], psn[:], Relu,
                        bias=bias_sb[:, 9 + m:10 + m],
                    )
            # X[code, :] = D2T^T @ Wd3 + bd3   (written to DRAM)
            wd3_sb = pws.tile([P, KC2, IN], f32, tag="bigw")
            nc.sync.dma_start(wd3_sb[:], Wd3[:].rearrange("(c p) m -> p c m", p=P))
            for cc in range(KCODES // P):
                xsb = pxs.tile([P, IN], f32, tag="xsb")
                for n in range(NXC):
                    psn = ps_m.tile([P, 512], f32, tag="psm")
                    for k in range(MC1):
                        nc.tensor.matmul(
                            psn[:, 0:XCW],
                            d2t[k][:, cc * P:(cc + 1) * P],
                            wd3_sb[:, k, n * XCW:(n + 1) * XCW],
                            start=(k == 0), stop=False,
                        )
                    # + bd3 broadcast via rank-1 ones matmul (exact fp32)
                    nc.tensor.matmul(
                        psn[:, 0:XCW],
                        ones_row[:],
                        bd3_sb[0:1, n * XCW:(n + 1) * XCW],
                        start=False, stop=True,
                    )
                    nc.scalar.copy(xsb[:, n * XCW:(n + 1) * XCW], psn[:, 0:XCW])
                xw = nc.sync.dma_start(Xd[cc * P:(cc + 1) * P, :], xsb[:])

            # ---------------- encoder weights (shared slot w/ wd3) -------
            we1_sb = pws.tile([P, KC1, H1], f32, tag="bigw")
            for c in range(KC1):
                kc = KC1_LAST if c == KC1 - 1 else P
                nc.sync.dma_start(we1_sb[0:kc, c, :], We1[c * P:c * P + kc, :])

            code_acc = psm.tile([P, NRT], u32, tag="code")

            # ---------------- streaming encoder over 8 blocks ------------
            for blk in range(NBLK):
                c0 = blk * BLK
                # x^T block, loaded in two half-chunk groups
                xt0 = pxt.tile([P, 8, BLK], f32, tag="xt0")
                nc.sync.dma_start(
                    xt0[:],
                    xT[0:1024, c0:c0 + BLK].rearrange("(c p) n -> p c n", p=P),
                )
                xt1 = pxt.tile([P, 8, BLK], f32, tag="xt1")
                nc.sync.dma_start(
                    xt1[:, 0:7, :],
                    xT[1024:1920, c0:c0 + BLK].rearrange("(c p) n -> p c n", p=P),
                )
                nc.sync.dma_start(xt1[0:KC1_LAST, 7, :], xT[1920:IN, c0:c0 + BLK])

                def xt_chunk(c):
                    t = xt0 if c < 8 else xt1
                    kc = KC1_LAST if c == KC1 - 1 else P
                    return t[0:kc, c % 8, :]

                # L1: h1T[h1, row] = relu(We1^T @ xT + be1)
                h1 = pact.tile([P, MC1, BLK], f32, tag="h1")
                for m in range(MC1):
                    ph = ps_h1.tile([P, BLK], f32, tag="ps1")
                    for c in range(KC1):
                        kc = KC1_LAST if c == KC1 - 1 else P
                        nc.tensor.matmul(
                            ph[:], we1_sb[0:kc, c, m * P:(m + 1) * P], xt_chunk(c),
                            start=(c == 0), stop=(c == KC1 - 1),
                        )
                    nc.scalar.activation(
                        h1[:, m, :], ph[:], Relu, bias=bias_sb[:, m:m + 1]
                    )
                # L2
                h2 = pact.tile([P, MC2, BLK], f32, tag="h2")
                for m in range(MC2):
                    ph = ps_h2.tile([P, BLK], f32, tag="ps2")
                    for k in range(KC2):
                        nc.tensor.matmul(
                            ph[:], we2_sb[:, k, m * P:(m + 1) * P], h1[:, k, :],
                            start=(k == 0), stop=(k == KC2 - 1),
                        )
                    nc.scalar.activation(
                        h2[:, m, :], ph[:], Relu, bias=bias_sb[:, 4 + m:5 + m]
                    )
                # L3 -> z'T  (rows 0:32 = z^T, row 32 = ones)
                zt = pact.tile([LAT + 1, BLK], f32, tag="zt")
                pz = ps_z.tile([LAT, BLK], f32, tag="psz")
                for k in range(KC3):
                    nc.tensor.matmul(
                        pz[:], we3_sb[:, k, :], h2[:, k, :],
                        start=(k == 0), stop=(k == KC3 - 1),
                    )
                nc.vector.tensor_scalar_add(zt[0:LAT, :], pz[:], bias_sb[0:LAT, 6:7])
                nc.gpsimd.memset(zt[LAT:LAT + 1, :], 1.0)
                nc.sync.dma_start(zeT_o[:, c0:c0 + BLK], zt[0:LAT, :])

                # scores + argmax + gather per 128-row tile
                for r in range(4):
                    rt = blk * 4 + r
                    sc = psc.tile([P, KCODES], f32, tag="sc")
                    for n in range(NSC):
                        pss = ps_sc.tile([P, 512], f32, tag="pssc")
                        nc.tensor.matmul(
                            pss[:],
                            zt[0:LAT + 1, r * P:(r + 1) * P],
                            e2t[0:LAT + 1, n * 512:(n + 1) * 512],
                            start=True, stop=True,
                        )
                        nc.scalar.copy(sc[:, n * 512:(n + 1) * 512], pss[:])
                    mx = psm.tile([P, 8], f32, tag="mx")
                    mi = psm.tile([P, 8], u32, tag="mi")
                    nc.vector.max(mx[:], sc[:])
                    nc.vector.max_index(mi[:], mx[:], sc[:])
                    nc.vector.tensor_copy(code_acc[:, rt:rt + 1], mi[:, 0:1])

                    gb = pg.tile([P, IN], f32, tag="gb")
                    nc.gpsimd.indirect_dma_start(
                        out=gb[:],
                        out_offset=None,
                        in_=Xd[:],
                        in_offset=IndirectOffsetOnAxis(
                            ap=code_acc[:, rt:rt + 1], axis=0
                        ),
                    )
                    nc.sync.dma_start(x_rec[rt * P:(rt + 1) * P, :], gb[:])

            nc.sync.dma_start(
                code_o[:].rearrange("(t p) -> p t", p=P), code_acc[:]
            )

    nc.compile()
    return nc


def _run_device(inputs):
    from concourse.bass_utils import run_bass_kernel_spmd

    x = np.ascontiguousarray(inputs["x"], dtype=np.float32)
    E = np.ascontiguousarray(inputs["E"], dtype=np.float32)
    ET = np.ascontiguousarray(E.T)

    shared = {
        "We1": np.ascontiguousarray(inputs["We1"], np.float32),
        "be1": np.ascontiguousarray(inputs["be1"], np.float32),
        "We2": np.ascontiguousarray(inputs["We2"], np.float32),
        "be2": np.ascontiguousarray(inputs["be2"], np.float32),
        "We3": np.ascontiguousarray(inputs["We3"], np.float32),
        "be3": np.ascontiguousarray(inputs["be3"], np.float32),
        "Wd1": np.ascontiguousarray(inputs["Wd1"], np.float32),
        "bd1": np.ascontiguousarray(inputs["bd1"], np.float32),
        "Wd2": np.ascontiguousarray(inputs["Wd2"], np.float32),
        "bd2": np.ascontiguousarray(inputs["bd2"], np.float32),
        "Wd3": np.ascontiguousarray(inputs["Wd3"], np.float32),
        "bd3": np.ascontiguousarray(inputs["bd3"], np.float32),
        "ET": ET,
    }
    in_maps = []
    for c in range(NCORES):
        m = dict(shared)
        m["xT"] = np.ascontiguousarray(x[c * S:(c + 1) * S].T)
        in_maps.append(m)

    if "nc" not in _CACHED:
        _CACHED["nc"] = _build_nc()
    nc = _CACHED["nc"]

    trace = bool(int(os.environ.get("KERNEL_TRACE", "0")))
    res = run_bass_kernel_spmd(
        nc, in_maps, core_ids=list(range(NCORES)), trace=trace
    )
    if trace and res.exec_time_ns:
        print(f"HW exec time: {res.exec_time_ns} ns")
        _CACHED["exec_time_ns"] = res.exec_time_ns
        _CACHED["trace"] = res.instructions_and_trace
    x_rec = np.concatenate([res.results[c]["x_rec"] for c in range(NCORES)], axis=0)
    code = np.concatenate(
        [res.results[c]["code"].astype(np.int32) for c in range(NCORES)]
    )
    z_e = np.concatenate(
        [np.ascontiguousarray(res.results[c]["zeT"].T) for c in range(NCORES)],
        axis=0,
    )
    return x_rec, code, z_e


def kernel(**inputs):
    x_rec, code, z_e = _run_device(inputs)

    E = np.asarray(inputs["E"], np.float32)
    z_q = E[code]
    d = (z_q.astype(np.float64) - z_e.astype(np.float64)) ** 2
    m = d.mean()
    vq_loss = np.float32(m + 0.25 * m)

    counts = np.bincount(code, minlength=KCODES).astype(np.float64)
    probs = counts / (counts.sum() + 1e-8)
    valid = probs > 0
    n_valid = float(valid.sum())
    safe_p = np.where(valid, probs, 1.0)
    usage_loss = np.float32(
        np.sum(np.where(valid, safe_p * np.log(safe_p * n_valid), 0.0))
    )

    return x_rec, vq_loss, usage_loss, code.astype(np.int32)


# revision 11
# speedup vs baseline: 1.1973x; 1.1973x over previous
"""Class-conditional VQ-VAE forward pass on 8 Trainium2 NeuronCores.

Strategy (data-parallel over batch, per spec sharding_hint):
  - Host shards the batch 8 ways, passes each core x^T (transposed shard) +
    replicated weights + E^T.
  - Device per core:
      * builds the full decoder lookup table X[1024, 2000] = decode(E) once
        (every batch row's decoder output is a function of its code only,
        since z_st == z_q numerically in the forward pass),
      * runs the encoder MLP in fp32 as chained transposed matmuls
        (zero on-chip transposes),
      * VQ argmin via scores s = 2*z.e - |e|^2 (one matmul with an appended
        ones-row; argmax == argmin of distances) + Max8/MaxIndex,
      * x_rec rows come from an indirect-DMA gather of X by code index.
  - Host assembles shards and computes the two scalar losses from
    z_e / code_idx (tiny reductions).
All matmuls are fp32 so code_idx matches the fp32 reference argmin exactly.
"""

import os
import numpy as np

B, IN, H1, H2, LAT, KCODES = 32768, 2000, 512, 256, 32, 1024
NCORES = 8
S = B // NCORES          # 4096 rows per core
BLK = 512                # rows per pipeline block
NBLK = S // BLK          # 8
NRT = S // 128           # 32 row-tiles per core
P = 128
KC1 = (IN + P - 1) // P  # 16 chunks over IN (15*128 + 80)
KC1_LAST = IN - (KC1 - 1) * P  # 80
KC2 = H1 // P            # 4
KC3 = H2 // P            # 2
MC1 = H1 // P            # 4 output chunks of L1
MC2 = H2 // P            # 2 output chunks of L2
NSC = KCODES // 512      # 2 score column chunks
NXC = 4                  # X free-dim chunks of 500
XCW = IN // NXC          # 500

_CACHED = {}


def _build_nc():
    import concourse.tile as tile
    import concourse.mybir as mybir
    from concourse import bacc

    f32 = mybir.dt.float32
    u32 = mybir.dt.uint32

    nc = bacc.Bacc(trn_type="TRN2")

    # ---- DRAM I/O ----
    f16 = mybir.dt.float16
    xTh = nc.dram_tensor("xTh", [IN, S], f16, kind="ExternalInput")
    xTl = nc.dram_tensor("xTl", [IN, S], f16, kind="ExternalInput")
    We1h = nc.dram_tensor("We1h", [IN, H1], f16, kind="ExternalInput")
    We1l = nc.dram_tensor("We1l", [IN, H1], f16, kind="ExternalInput")
    be1 = nc.dram_tensor("be1", [H1], f32, kind="ExternalInput")
    We2 = nc.dram_tensor("We2", [H1, H2], f32, kind="ExternalInput")
    be2 = nc.dram_tensor("be2", [H2], f32, kind="ExternalInput")
    We3 = nc.dram_tensor("We3", [H2, LAT], f32, kind="ExternalInput")
    be3 = nc.dram_tensor("be3", [LAT], f32, kind="ExternalInput")
    Wd1 = nc.dram_tensor("Wd1", [LAT, H2], f32, kind="ExternalInput")
    bd1 = nc.dram_tensor("bd1", [H2], f32, kind="ExternalInput")
    Wd2 = nc.dram_tensor("Wd2", [H2, H1], f32, kind="ExternalInput")
    bd2 = nc.dram_tensor("bd2", [H1], f32, kind="ExternalInput")
    Wd3 = nc.dram_tensor("Wd3", [H1, IN], f32, kind="ExternalInput")
    ETd = nc.dram_tensor("ET", [LAT, KCODES], f32, kind="ExternalInput")

    x_rec = nc.dram_tensor("x_rec", [S, IN], f32, kind="ExternalOutput")
    code_o = nc.dram_tensor("code", [S], u32, kind="ExternalOutput")
    zeT_o = nc.dram_tensor("zeT", [LAT, S], f32, kind="ExternalOutput")

    Xd = nc.dram_tensor("Xtab", [KCODES, IN], f32, kind="Internal")

    Relu = mybir.ActivationFunctionType.Relu
    Copy = mybir.ActivationFunctionType.Copy
    from concourse.bass import IndirectOffsetOnAxis

    with tile.TileContext(nc) as tc:
        with (
            tc.tile_pool(name="wshare", bufs=1) as pws,   # we1/wd3 shared slot
            tc.tile_pool(name="wsmall", bufs=1) as pw,
            tc.tile_pool(name="dec", bufs=1) as pdec,
            tc.tile_pool(name="xstage", bufs=1) as pxs,
            tc.tile_pool(name="xt", bufs=2) as pxt,
            tc.tile_pool(name="act", bufs=2) as pact,
            tc.tile_pool(name="scr", bufs=2) as psc,
            tc.tile_pool(name="small", bufs=1) as psm,
            tc.tile_pool(name="gat", bufs=2) as pg,
            tc.tile_pool(name="ps_h1", bufs=2, space="PSUM") as ps_h1,
            tc.tile_pool(name="ps_h2", bufs=1, space="PSUM") as ps_h2,
            tc.tile_pool(name="ps_z", bufs=1, space="PSUM") as ps_z,
            tc.tile_pool(name="ps_sc", bufs=2, space="PSUM") as ps_sc,
            tc.tile_pool(name="ps_m", bufs=2, space="PSUM") as ps_m,
        ):
            # ---------------- weight/bias loads (small ones) -------------
            we2_sb = pw.tile([P, KC2, H2], f32, tag="we2")
            nc.sync.dma_start(we2_sb[:], We2[:].rearrange("(c p) m -> p c m", p=P))
            we3_sb = pw.tile([P, KC3, LAT], f32, tag="we3")
            nc.sync.dma_start(we3_sb[:], We3[:].rearrange("(c p) m -> p c m", p=P))
            wd1_sb = pw.tile([LAT, H2], f32, tag="wd1")
            nc.sync.dma_start(wd1_sb[:], Wd1[:])
            wd2_sb = pw.tile([P, KC3, H1], f32, tag="wd2")
            nc.sync.dma_start(wd2_sb[:], Wd2[:].rearrange("(c p) m -> p c m", p=P))

            bias_sb = pw.tile([P, 16], f32, tag="bias")
            nc.sync.dma_start(bias_sb[:, 0:4], be1[:].rearrange("(c p) -> p c", p=P))
            nc.sync.dma_start(bias_sb[:, 4:6], be2[:].rearrange("(c p) -> p c", p=P))
            nc.sync.dma_start(bias_sb[0:LAT, 6:7], be3[:, None])
            nc.sync.dma_start(bias_sb[:, 7:9], bd1[:].rearrange("(c p) -> p c", p=P))
            nc.sync.dma_start(bias_sb[:, 9:13], bd2[:].rearrange("(c p) -> p c", p=P))

            et_sb = pw.tile([LAT, KCODES], f32, tag="et")
            nc.sync.dma_start(et_sb[:], ETd[:])

            # ---------------- VQ score matrix  E2t = [2*E^T ; -|e|^2] ----
            e2t = pw.tile([64, KCODES], f32, tag="e2t")
            nc.vector.tensor_scalar_mul(e2t[0:LAT, :], et_sb[:], 2.0)
            ones32 = pw.tile([LAT, 1], f32, tag="ones")
            nc.gpsimd.memset(ones32[:], 1.0)
            # squared codebook: use d1t[0] rows 0:32 as scratch (WAR-safe,
            # D1T is written only after the colsum matmuls below read it)
            d1t = [pdec.tile([P, KCODES], f32, tag=f"d1_{m}", name=f"d1_{m}") for m in range(MC2)]
            sq = d1t[0][0:LAT, :]
            nc.vector.tensor_mul(sq, et_sb[:], et_sb[:])
            for n in range(NSC):
                psn = ps_m.tile([P, 512], f32, tag="psm")
                nc.tensor.matmul(
                    psn[0:1, :], ones32[:], sq[:, n * 512:(n + 1) * 512],
                    start=True, stop=True,
                )
                nc.scalar.activation(
                    e2t[LAT:LAT + 1, n * 512:(n + 1) * 512], psn[0:1, :],
                    Copy, scale=-1.0,
                )

            # ---------------- decoder table X = decode(E) ----------------
            # D1T[h2, code] = relu(Wd1^T @ E^T + bd1)
            for m in range(MC2):
                for n in range(NSC):
                    psn = ps_m.tile([P, 512], f32, tag="psm")
                    nc.tensor.matmul(
                        psn[:], wd1_sb[:, m * P:(m + 1) * P],
                        et_sb[:, n * 512:(n + 1) * 512],
                        start=True, stop=True,
                    )
                    nc.scalar.activation(
                        d1t[m][:, n * 512:(n + 1) * 512], psn[:], Relu,
                        bias=bias_sb[:, 7 + m:8 + m],
                    )
            # D2T[h1, code] = relu(Wd2^T @ D1T + bd2)
            d2t = [pdec.tile([P, KCODES], f32, tag=f"d2_{m}", name=f"d2_{m}") for m in range(MC1)]
            for m in range(MC1):
                for n in range(NSC):
                    psn = ps_m.tile([P, 512], f32, tag="psm")
                    for k in range(KC3):
                        nc.tensor.matmul(
                            psn[:], wd2_sb[:, k, m * P:(m + 1) * P],
                            d1t[k][:, n * 512:(n + 1) * 512],
                            start=(k == 0), stop=(k == KC3 - 1),
                        )
                    nc.scalar.activation(
                        d2t[m][:, n * 512:(n + 1) * 512], psn[:], Relu,
                        bias=bias_sb[:, 9 + m:10 + m],
                    )
            # X[code, :] = D2T^T @ Wd3 + bd3   (written to DRAM)
            wd3_sb = pws.tile([P, KC2, IN], f32, tag="bigw")
            nc.sync.dma_start(wd3_sb[:], Wd3[:].rearrange("(c p) m -> p c m", p=P))
            for cc in range(KCODES // P):
                xsb = pxs.tile([P, IN], f32, tag="xsb")
                for n in range(NXC):
                    psn = ps_m.tile([P, 512], f32, tag="psm")
                    for k in range(MC1):
                        nc.tensor.matmul(
                            psn[:, 0:XCW],
                            d2t[k][:, cc * P:(cc + 1) * P],
                            wd3_sb[:, k, n * XCW:(n + 1) * XCW],
                            start=(k == 0), stop=(k == MC1 - 1),
                        )
                    nc.scalar.copy(xsb[:, n * XCW:(n + 1) * XCW], psn[:, 0:XCW])
                xw = nc.sync.dma_start(Xd[cc * P:(cc + 1) * P, :], xsb[:])

            # ---------------- encoder weights (shared slot w/ wd3) -------
            we1_sb = pws.tile([P, KC1, 2, H1], f16, tag="bigw")
            for c in range(KC1):
                kc = KC1_LAST if c == KC1 - 1 else P
                nc.sync.dma_start(we1_sb[0:kc, c, 0, :], We1h[c * P:c * P + kc, :])
                nc.sync.dma_start(we1_sb[0:kc, c, 1, :], We1l[c * P:c * P + kc, :])

            code_acc = psm.tile([P, NRT], u32, tag="code")

            # ---------------- streaming encoder over 8 blocks ------------
            for blk in range(NBLK):
                c0 = blk * BLK
                # x^T block, loaded in two half-chunk groups
                xts = []
                for hl, src in (("h", xTh), ("l", xTl)):
                    xt0 = pxt.tile([P, 8, BLK], f16, tag=f"xt0{hl}", name=f"xt0{hl}")
                    nc.sync.dma_start(
                        xt0[:],
                        src[0:1024, c0:c0 + BLK].rearrange("(c p) n -> p c n", p=P),
                    )
                    xt1 = pxt.tile([P, 8, BLK], f16, tag=f"xt1{hl}", name=f"xt1{hl}")
                    nc.sync.dma_start(
                        xt1[:, 0:7, :],
                        src[1024:1920, c0:c0 + BLK].rearrange("(c p) n -> p c n", p=P),
                    )
                    nc.sync.dma_start(xt1[0:KC1_LAST, 7, :], src[1920:IN, c0:c0 + BLK])
                    xts.append((xt0, xt1))

                def xt_chunk(c, hl):
                    t = xts[hl][0] if c < 8 else xts[hl][1]
                    kc = KC1_LAST if c == KC1 - 1 else P
                    return t[0:kc, c % 8, :]

                # L1: h1T[h1, row] = relu(We1^T @ xT + be1)
                h1 = pact.tile([P, MC1, BLK], f32, tag="h1")
                for m in range(MC1):
                    ph = ps_h1.tile([P, BLK], f32, tag="ps1")
                    for c in range(KC1):
                        kc = KC1_LAST if c == KC1 - 1 else P
                        for j, (wj, xj) in enumerate(((0, 0), (0, 1), (1, 0))):
                            nc.tensor.matmul(
                                ph[:],
                                we1_sb[0:kc, c, wj, m * P:(m + 1) * P],
                                xt_chunk(c, xj),
                                start=(c == 0 and j == 0),
                                stop=(c == KC1 - 1 and j == 2),
                            )
                    nc.scalar.activation(
                        h1[:, m, :], ph[:], Relu, bias=bias_sb[:, m:m + 1]
                    )
                # L2
                h2 = pact.tile([P, MC2, BLK], f32, tag="h2")
                for m in range(MC2):
                    ph = ps_h2.tile([P, BLK], f32, tag="ps2")
                    for k in range(KC2):
                        nc.tensor.matmul(
                            ph[:], we2_sb[:, k, m * P:(m + 1) * P], h1[:, k, :],
                            start=(k == 0), stop=(k == KC2 - 1),
                        )
                    nc.scalar.activation(
                        h2[:, m, :], ph[:], Relu, bias=bias_sb[:, 4 + m:5 + m]
                    )
                # L3 -> z'T  (rows 0:32 = z^T, row 32 = ones)
                zt = pact.tile([LAT + 1, BLK], f32, tag="zt")
                pz = ps_z.tile([LAT, BLK], f32, tag="psz")
                for k in range(KC3):
                    nc.tensor.matmul(
                        pz[:], we3_sb[:, k, :], h2[:, k, :],
                        start=(k == 0), stop=(k == KC3 - 1),
                    )
                nc.vector.tensor_scalar_add(zt[0:LAT, :], pz[:], bias_sb[0:LAT, 6:7])
                nc.gpsimd.memset(zt[LAT:LAT + 1, :], 1.0)
                nc.sync.dma_start(zeT_o[:, c0:c0 + BLK], zt[0:LAT, :])

                # scores + argmax + gather per 128-row tile
                for r in range(4):
                    rt = blk * 4 + r
                    sc = psc.tile([P, KCODES], f32, tag="sc")
                    for n in range(NSC):
                        pss = ps_sc.tile([P, 512], f32, tag="pssc")
                        nc.tensor.matmul(
                            pss[:],
                            zt[0:LAT + 1, r * P:(r + 1) * P],
                            e2t[0:LAT + 1, n * 512:(n + 1) * 512],
                            start=True, stop=True,
                        )
                        nc.scalar.copy(sc[:, n * 512:(n + 1) * 512], pss[:])
                    mx = psm.tile([P, 8], f32, tag="mx")
                    mi = psm.tile([P, 8], u32, tag="mi")
                    nc.vector.max(mx[:], sc[:])
                    nc.vector.max_index(mi[:], mx[:], sc[:])
                    nc.vector.tensor_copy(code_acc[:, rt:rt + 1], mi[:, 0:1])

                    gb = pg.tile([P, IN], f32, tag="gb")
                    nc.gpsimd.indirect_dma_start(
                        out=gb[:],
                        out_offset=None,
                        in_=Xd[:],
                        in_offset=IndirectOffsetOnAxis(
                            ap=code_acc[:, rt:rt + 1], axis=0
                        ),
                    )
                    nc.sync.dma_start(x_rec[rt * P:(rt + 1) * P, :], gb[:])

            nc.sync.dma_start(
                code_o[:].rearrange("(t p) -> p t", p=P), code_acc[:]
            )

    nc.compile()
    return nc


def _run_device(inputs):
    from concourse.bass_utils import run_bass_kernel_spmd

    x = np.ascontiguousarray(inputs["x"], dtype=np.float32)
    E = np.ascontiguousarray(inputs["E"], dtype=np.float32)
    ET = np.ascontiguousarray(E.T)

    We1 = np.ascontiguousarray(inputs["We1"], np.float32)
    We1h = We1.astype(np.float16)
    We1l = (We1 - We1h.astype(np.float32)).astype(np.float16)
    shared = {
        "We1h": We1h,
        "We1l": We1l,
        "be1": np.ascontiguousarray(inputs["be1"], np.float32),
        "We2": np.ascontiguousarray(inputs["We2"], np.float32),
        "be2": np.ascontiguousarray(inputs["be2"], np.float32),
        "We3": np.ascontiguousarray(inputs["We3"], np.float32),
        "be3": np.ascontiguousarray(inputs["be3"], np.float32),
        "Wd1": np.ascontiguousarray(inputs["Wd1"], np.float32),
        "bd1": np.ascontiguousarray(inputs["bd1"], np.float32),
        "Wd2": np.ascontiguousarray(inputs["Wd2"], np.float32),
        "bd2": np.ascontiguousarray(inputs["bd2"], np.float32),
        "Wd3": np.ascontiguousarray(inputs["Wd3"], np.float32),
        "ET": ET,
    }
    bd3 = np.ascontiguousarray(inputs["bd3"], np.float32)
    in_maps = []
    for c in range(NCORES):
        m = dict(shared)
        xt = np.ascontiguousarray(x[c * S:(c + 1) * S].T)
        xh = xt.astype(np.float16)
        m["xTh"] = xh
        m["xTl"] = (xt - xh.astype(np.float32)).astype(np.float16)
        in_maps.append(m)

    if "nc" not in _CACHED:
        _CACHED["nc"] = _build_nc()
    nc = _CACHED["nc"]

    trace = bool(int(os.environ.get("KERNEL_TRACE", "0")))
    res = run_bass_kernel_spmd(
        nc, in_maps, core_ids=list(range(NCORES)), trace=trace
    )
    if trace and res.exec_time_ns:
        print(f"HW exec time: {res.exec_time_ns} ns")
        _CACHED["exec_time_ns"] = res.exec_time_ns
        _CACHED["trace"] = res.instructions_and_trace
    x_rec = np.concatenate([res.results[c]["x_rec"] for c in range(NCORES)], axis=0)
    x_rec += bd3[None, :]
    code = np.concatenate(
        [res.results[c]["code"].astype(np.int32) for c in range(NCORES)]
    )
    z_e = np.concatenate(
        [np.ascontiguousarray(res.results[c]["zeT"].T) for c in range(NCORES)],
        axis=0,
    )
    return x_rec, code, z_e


def kernel(**inputs):
    x_rec, code, z_e = _run_device(inputs)

    E = np.asarray(inputs["E"], np.float32)
    z_q = E[code]
    d = (z_q.astype(np.float64) - z_e.astype(np.float64)) ** 2
    m = d.mean()
    vq_loss = np.float32(m + 0.25 * m)

    counts = np.bincount(code, minlength=KCODES).astype(np.float64)
    probs = counts / (counts.sum() + 1e-8)
    valid = probs > 0
    n_valid = float(valid.sum())
    safe_p = np.where(valid, probs, 1.0)
    usage_loss = np.float32(
        np.sum(np.where(valid, safe_p * np.log(safe_p * n_valid), 0.0))
    )

    return x_rec, vq_loss, usage_loss, code.astype(np.int32)


# revision 12
# speedup vs baseline: 1.2407x; 1.0362x over previous
"""Class-conditional VQ-VAE forward pass on 8 Trainium2 NeuronCores.

Strategy (data-parallel over batch, per spec sharding_hint):
  - Host shards the batch 8 ways, passes each core x^T (transposed shard) +
    replicated weights + E^T.
  - Device per core:
      * builds the full decoder lookup table X[1024, 2000] = decode(E) once
        (every batch row's decoder output is a function of its code only,
        since z_st == z_q numerically in the forward pass),
      * runs the encoder MLP in fp32 as chained transposed matmuls
        (zero on-chip transposes),
      * VQ argmin via scores s = 2*z.e - |e|^2 (one matmul with an appended
        ones-row; argmax == argmin of distances) + Max8/MaxIndex,
      * x_rec rows come from an indirect-DMA gather of X by code index.
  - Host assembles shards and computes the two scalar losses from
    z_e / code_idx (tiny reductions).
All matmuls are fp32 so code_idx matches the fp32 reference argmin exactly.
"""

import os
import numpy as np

B, IN, H1, H2, LAT, KCODES = 32768, 2000, 512, 256, 32, 1024
NCORES = 8
S = B // NCORES          # 4096 rows per core
BLK = 512                # rows per pipeline block
NBLK = S // BLK          # 8
NRT = S // 128           # 32 row-tiles per core
P = 128
KC1 = (IN + P - 1) // P  # 16 chunks over IN (15*128 + 80)
KC1_LAST = IN - (KC1 - 1) * P  # 80
KC2 = H1 // P            # 4
KC3 = H2 // P            # 2
MC1 = H1 // P            # 4 output chunks of L1
MC2 = H2 // P            # 2 output chunks of L2
NSC = KCODES // 512      # 2 score column chunks
NXC = 4                  # X free-dim chunks of 500
XCW = IN // NXC          # 500

_CACHED = {}


def _build_nc():
    import concourse.tile as tile
    import concourse.mybir as mybir
    from concourse import bacc

    f32 = mybir.dt.float32
    u32 = mybir.dt.uint32

    nc = bacc.Bacc(trn_type="TRN2")

    # ---- DRAM I/O ----
    f16 = mybir.dt.float16
    xTh = nc.dram_tensor("xTh", [IN, S], f16, kind="ExternalInput")
    xTl = nc.dram_tensor("xTl", [IN, S], f16, kind="ExternalInput")
    We1h = nc.dram_tensor("We1h", [IN, H1], f16, kind="ExternalInput")
    We1l = nc.dram_tensor("We1l", [IN, H1], f16, kind="ExternalInput")
    be1 = nc.dram_tensor("be1", [H1], f32, kind="ExternalInput")
    We2 = nc.dram_tensor("We2", [H1, H2], f32, kind="ExternalInput")
    be2 = nc.dram_tensor("be2", [H2], f32, kind="ExternalInput")
    We3 = nc.dram_tensor("We3", [H2, LAT], f32, kind="ExternalInput")
    be3 = nc.dram_tensor("be3", [LAT], f32, kind="ExternalInput")
    Wd1 = nc.dram_tensor("Wd1", [LAT, H2], f32, kind="ExternalInput")
    bd1 = nc.dram_tensor("bd1", [H2], f32, kind="ExternalInput")
    Wd2 = nc.dram_tensor("Wd2", [H2, H1], f32, kind="ExternalInput")
    bd2 = nc.dram_tensor("bd2", [H1], f32, kind="ExternalInput")
    Wd3 = nc.dram_tensor("Wd3", [H1, IN], f32, kind="ExternalInput")
    ETd = nc.dram_tensor("ET", [LAT, KCODES], f32, kind="ExternalInput")

    x_rec = nc.dram_tensor("x_rec", [S, IN], f32, kind="ExternalOutput")
    code_o = nc.dram_tensor("code", [S], u32, kind="ExternalOutput")
    zeT_o = nc.dram_tensor("zeT", [LAT, S], f32, kind="ExternalOutput")

    Xd = nc.dram_tensor("Xtab", [KCODES, IN], f32, kind="Internal")

    Relu = mybir.ActivationFunctionType.Relu
    Copy = mybir.ActivationFunctionType.Copy
    from concourse.bass import IndirectOffsetOnAxis

    with tile.TileContext(nc) as tc:
        with (
            tc.tile_pool(name="wshare", bufs=1) as pws,   # we1/wd3 shared slot
            tc.tile_pool(name="wsmall", bufs=1) as pw,
            tc.tile_pool(name="dec", bufs=1) as pdec,
            tc.tile_pool(name="xstage", bufs=1) as pxs,
            tc.tile_pool(name="xt", bufs=2) as pxt,
            tc.tile_pool(name="act", bufs=2) as pact,
            tc.tile_pool(name="scr", bufs=2) as psc,
            tc.tile_pool(name="small", bufs=1) as psm,
            tc.tile_pool(name="gat", bufs=2) as pg,
            tc.tile_pool(name="ps_h1", bufs=2, space="PSUM") as ps_h1,
            tc.tile_pool(name="ps_h2", bufs=1, space="PSUM") as ps_h2,
            tc.tile_pool(name="ps_z", bufs=1, space="PSUM") as ps_z,
            tc.tile_pool(name="ps_sc", bufs=2, space="PSUM") as ps_sc,
            tc.tile_pool(name="ps_m", bufs=2, space="PSUM") as ps_m,
        ):
            # ---------------- weight/bias loads (small ones) -------------
            we2_sb = pw.tile([P, KC2, H2], f32, tag="we2")
            nc.sync.dma_start(we2_sb[:], We2[:].rearrange("(c p) m -> p c m", p=P))
            we3_sb = pw.tile([P, KC3, LAT], f32, tag="we3")
            nc.sync.dma_start(we3_sb[:], We3[:].rearrange("(c p) m -> p c m", p=P))
            wd1_sb = pw.tile([LAT, H2], f32, tag="wd1")
            nc.sync.dma_start(wd1_sb[:], Wd1[:])
            wd2_sb = pw.tile([P, KC3, H1], f32, tag="wd2")
            nc.sync.dma_start(wd2_sb[:], Wd2[:].rearrange("(c p) m -> p c m", p=P))

            bias_sb = pw.tile([P, 16], f32, tag="bias")
            nc.sync.dma_start(bias_sb[:, 0:4], be1[:].rearrange("(c p) -> p c", p=P))
            nc.sync.dma_start(bias_sb[:, 4:6], be2[:].rearrange("(c p) -> p c", p=P))
            nc.sync.dma_start(bias_sb[0:LAT, 6:7], be3[:, None])
            nc.sync.dma_start(bias_sb[:, 7:9], bd1[:].rearrange("(c p) -> p c", p=P))
            nc.sync.dma_start(bias_sb[:, 9:13], bd2[:].rearrange("(c p) -> p c", p=P))

            et_sb = pw.tile([LAT, KCODES], f32, tag="et")
            nc.sync.dma_start(et_sb[:], ETd[:])

            # ---------------- VQ score matrix  E2t = [2*E^T ; -|e|^2] ----
            e2t = pw.tile([P, KCODES], f32, tag="e2t")
            nc.vector.tensor_scalar_mul(e2t[0:LAT, :], et_sb[:], 2.0)
            ones32 = pw.tile([LAT, 1], f32, tag="ones")
            nc.gpsimd.memset(ones32[:], 1.0)
            # squared codebook: use d1t[0] rows 0:32 as scratch (WAR-safe,
            # D1T is written only after the colsum matmuls below read it)
            d1t = [pdec.tile([P, KCODES], f32, tag=f"d1_{m}", name=f"d1_{m}") for m in range(MC2)]
            sq = d1t[0][0:LAT, :]
            nc.vector.tensor_mul(sq, et_sb[:], et_sb[:])
            for n in range(NSC):
                psn = ps_m.tile([P, 512], f32, tag="psm")
                nc.tensor.matmul(
                    psn[0:1, :], ones32[:], sq[:, n * 512:(n + 1) * 512],
                    start=True, stop=True,
                )
                nc.scalar.activation(
                    e2t[LAT:LAT + 1, n * 512:(n + 1) * 512], psn[0:1, :],
                    Copy, scale=-1.0,
                )

            # duplicate score matrix at partition 64 for 2-group PE packing
            nc.sync.dma_start(e2t[64:64 + LAT + 1, :], e2t[0:LAT + 1, :])

            # ---------------- decoder table X = decode(E) ----------------
            # D1T[h2, code] = relu(Wd1^T @ E^T + bd1)
            for m in range(MC2):
                for n in range(NSC):
                    psn = ps_m.tile([P, 512], f32, tag="psm")
                    nc.tensor.matmul(
                        psn[:], wd1_sb[:, m * P:(m + 1) * P],
                        et_sb[:, n * 512:(n + 1) * 512],
                        start=True, stop=True,
                    )
                    nc.scalar.activation(
                        d1t[m][:, n * 512:(n + 1) * 512], psn[:], Relu,
                        bias=bias_sb[:, 7 + m:8 + m],
                    )
            # D2T[h1, code] = relu(Wd2^T @ D1T + bd2)
            d2t = [pdec.tile([P, KCODES], f32, tag=f"d2_{m}", name=f"d2_{m}") for m in range(MC1)]
            for m in range(MC1):
                for n in range(NSC):
                    psn = ps_m.tile([P, 512], f32, tag="psm")
                    for k in range(KC3):
                        nc.tensor.matmul(
                            psn[:], wd2_sb[:, k, m * P:(m + 1) * P],
                            d1t[k][:, n * 512:(n + 1) * 512],
                            start=(k == 0), stop=(k == KC3 - 1),
                        )
                    nc.scalar.activation(
                        d2t[m][:, n * 512:(n + 1) * 512], psn[:], Relu,
                        bias=bias_sb[:, 9 + m:10 + m],
                    )
            # X[code, :] = D2T^T @ Wd3 + bd3   (written to DRAM)
            wd3_sb = pws.tile([P, KC2, IN], f32, tag="bigw")
            nc.sync.dma_start(wd3_sb[:], Wd3[:].rearrange("(c p) m -> p c m", p=P))
            for cc in range(KCODES // P):
                xsb = pxs.tile([P, IN], f32, tag="xsb")
                for n in range(NXC):
                    psn = ps_m.tile([P, 512], f32, tag="psm")
                    for k in range(MC1):
                        nc.tensor.matmul(
                            psn[:, 0:XCW],
                            d2t[k][:, cc * P:(cc + 1) * P],
                            wd3_sb[:, k, n * XCW:(n + 1) * XCW],
                            start=(k == 0), stop=(k == MC1 - 1),
                        )
                    nc.scalar.copy(xsb[:, n * XCW:(n + 1) * XCW], psn[:, 0:XCW])
                xw = nc.sync.dma_start(Xd[cc * P:(cc + 1) * P, :], xsb[:])

            # ---------------- encoder weights (shared slot w/ wd3) -------
            we1_sb = pws.tile([P, KC1, 2, H1], f16, tag="bigw")
            for c in range(KC1):
                kc = KC1_LAST if c == KC1 - 1 else P
                nc.sync.dma_start(we1_sb[0:kc, c, 0, :], We1h[c * P:c * P + kc, :])
                nc.sync.dma_start(we1_sb[0:kc, c, 1, :], We1l[c * P:c * P + kc, :])

            code_acc = psm.tile([P, NRT], u32, tag="code")

            # ---------------- streaming encoder over 8 blocks ------------
            for blk in range(NBLK):
                c0 = blk * BLK
                # x^T block, loaded in two half-chunk groups
                xts = []
                for hl, src in (("h", xTh), ("l", xTl)):
                    xt0 = pxt.tile([P, 8, BLK], f16, tag=f"xt0{hl}", name=f"xt0{hl}")
                    nc.sync.dma_start(
                        xt0[:],
                        src[0:1024, c0:c0 + BLK].rearrange("(c p) n -> p c n", p=P),
                    )
                    xt1 = pxt.tile([P, 8, BLK], f16, tag=f"xt1{hl}", name=f"xt1{hl}")
                    nc.sync.dma_start(
                        xt1[:, 0:7, :],
                        src[1024:1920, c0:c0 + BLK].rearrange("(c p) n -> p c n", p=P),
                    )
                    nc.sync.dma_start(xt1[0:KC1_LAST, 7, :], src[1920:IN, c0:c0 + BLK])
                    xts.append((xt0, xt1))

                def xt_chunk(c, hl):
                    t = xts[hl][0] if c < 8 else xts[hl][1]
                    kc = KC1_LAST if c == KC1 - 1 else P
                    return t[0:kc, c % 8, :]

                # L1: h1T[h1, row] = relu(We1^T @ xT + be1)
                h1 = pact.tile([P, MC1, BLK], f32, tag="h1")
                for m in range(MC1):
                    ph = ps_h1.tile([P, BLK], f32, tag="ps1")
                    for c in range(KC1):
                        kc = KC1_LAST if c == KC1 - 1 else P
                        for j, (wj, xj) in enumerate(((0, 0), (0, 1), (1, 0))):
                            nc.tensor.matmul(
                                ph[:],
                                we1_sb[0:kc, c, wj, m * P:(m + 1) * P],
                                xt_chunk(c, xj),
                                start=(c == 0 and j == 0),
                                stop=(c == KC1 - 1 and j == 2),
                            )
                    nc.scalar.activation(
                        h1[:, m, :], ph[:], Relu, bias=bias_sb[:, m:m + 1]
                    )
                # L2
                h2 = pact.tile([P, MC2, BLK], f32, tag="h2")
                for m in range(MC2):
                    ph = ps_h2.tile([P, BLK], f32, tag="ps2")
                    for k in range(KC2):
                        nc.tensor.matmul(
                            ph[:], we2_sb[:, k, m * P:(m + 1) * P], h1[:, k, :],
                            start=(k == 0), stop=(k == KC2 - 1),
                        )
                    nc.scalar.activation(
                        h2[:, m, :], ph[:], Relu, bias=bias_sb[:, 4 + m:5 + m]
                    )
                # L3 -> z'T  (rows 0:32 = z^T, row 32 = ones)
                zt = pact.tile([P, BLK], f32, tag="zt")
                pz = ps_z.tile([LAT, BLK], f32, tag="psz")
                for k in range(KC3):
                    nc.tensor.matmul(
                        pz[:], we3_sb[:, k, :], h2[:, k, :],
                        start=(k == 0), stop=(k == KC3 - 1),
                    )
                nc.vector.tensor_scalar_add(zt[0:LAT, :], pz[:], bias_sb[0:LAT, 6:7])
                nc.gpsimd.memset(zt[LAT:LAT + 1, :], 1.0)
                nc.sync.dma_start(zt[64:64 + LAT + 1, :], zt[0:LAT + 1, :])
                nc.sync.dma_start(zeT_o[:, c0:c0 + BLK], zt[0:LAT, :])

                # scores + argmax + gather per 128-row tile
                for r in range(4):
                    rt = blk * 4 + r
                    b0 = 64 * (r % 2)
                    sc = psc.tile([P, KCODES], f32, tag="sc")
                    for n in range(NSC):
                        pss = ps_sc.tile([P, 512], f32, tag="pssc")
                        nc.tensor.matmul(
                            pss[:],
                            zt[b0:b0 + LAT + 1, r * P:(r + 1) * P],
                            e2t[b0:b0 + LAT + 1, n * 512:(n + 1) * 512],
                            start=True, stop=True,
                        )
                        nc.scalar.copy(sc[:, n * 512:(n + 1) * 512], pss[:])
                    mx = psm.tile([P, 8], f32, tag="mx")
                    mi = psm.tile([P, 8], u32, tag="mi")
                    nc.vector.max(mx[:], sc[:])
                    nc.vector.max_index(mi[:], mx[:], sc[:])
                    nc.vector.tensor_copy(code_acc[:, rt:rt + 1], mi[:, 0:1])

                    gb = pg.tile([P, IN], f32, tag="gb")
                    nc.gpsimd.indirect_dma_start(
                        out=gb[:],
                        out_offset=None,
                        in_=Xd[:],
                        in_offset=IndirectOffsetOnAxis(
                            ap=code_acc[:, rt:rt + 1], axis=0
                        ),
                    )
                    nc.sync.dma_start(x_rec[rt * P:(rt + 1) * P, :], gb[:])

            nc.sync.dma_start(
                code_o[:].rearrange("(t p) -> p t", p=P), code_acc[:]
            )

    nc.compile()
    return nc


def _run_device(inputs):
    from concourse.bass_utils import run_bass_kernel_spmd

    x = np.ascontiguousarray(inputs["x"], dtype=np.float32)
    E = np.ascontiguousarray(inputs["E"], dtype=np.float32)
    ET = np.ascontiguousarray(E.T)

    We1 = np.ascontiguousarray(inputs["We1"], np.float32)
    We1h = We1.astype(np.float16)
    We1l = (We1 - We1h.astype(np.float32)).astype(np.float16)
    shared = {
        "We1h": We1h,
        "We1l": We1l,
        "be1": np.ascontiguousarray(inputs["be1"], np.float32),
        "We2": np.ascontiguousarray(inputs["We2"], np.float32),
        "be2": np.ascontiguousarray(inputs["be2"], np.float32),
        "We3": np.ascontiguousarray(inputs["We3"], np.float32),
        "be3": np.ascontiguousarray(inputs["be3"], np.float32),
        "Wd1": np.ascontiguousarray(inputs["Wd1"], np.float32),
        "bd1": np.ascontiguousarray(inputs["bd1"], np.float32),
        "Wd2": np.ascontiguousarray(inputs["Wd2"], np.float32),
        "bd2": np.ascontiguousarray(inputs["bd2"], np.float32),
        "Wd3": np.ascontiguousarray(inputs["Wd3"], np.float32),
        "ET": ET,
    }
    bd3 = np.ascontiguousarray(inputs["bd3"], np.float32)
    in_maps = []
    for c in range(NCORES):
        m = dict(shared)
        xt = np.ascontiguousarray(x[c * S:(c + 1) * S].T)
        xh = xt.astype(np.float16)
        m["xTh"] = xh
        m["xTl"] = (xt - xh.astype(np.float32)).astype(np.float16)
        in_maps.append(m)

    if "nc" not in _CACHED:
        _CACHED["nc"] = _build_nc()
    nc = _CACHED["nc"]

    trace = bool(int(os.environ.get("KERNEL_TRACE", "0")))
    res = run_bass_kernel_spmd(
        nc, in_maps, core_ids=list(range(NCORES)), trace=trace
    )
    if trace and res.exec_time_ns:
        print(f"HW exec time: {res.exec_time_ns} ns")
        _CACHED["exec_time_ns"] = res.exec_time_ns
        _CACHED["trace"] = res.instructions_and_trace
    x_rec = np.concatenate([res.results[c]["x_rec"] for c in range(NCORES)], axis=0)
    x_rec += bd3[None, :]
    code = np.concatenate(
        [res.results[c]["code"].astype(np.int32) for c in range(NCORES)]
    )
    z_e = np.concatenate(
        [np.ascontiguousarray(res.results[c]["zeT"].T) for c in range(NCORES)],
        axis=0,
    )
    return x_rec, code, z_e


def kernel(**inputs):
    x_rec, code, z_e = _run_device(inputs)

    E = np.asarray(inputs["E"], np.float32)
    z_q = E[code]
    d = (z_q.astype(np.float64) - z_e.astype(np.float64)) ** 2
    m = d.mean()
    vq_loss = np.float32(m + 0.25 * m)

    counts = np.bincount(code, minlength=KCODES).astype(np.float64)
    probs = counts / (counts.sum() + 1e-8)
    valid = probs > 0
    n_valid = float(valid.sum())
    safe_p = np.where(valid, probs, 1.0)
    usage_loss = np.float32(
        np.sum(np.where(valid, safe_p * np.log(safe_p * n_valid), 0.0))
    )

    return x_rec, vq_loss, usage_loss, code.astype(np.int32)
